# revision 1
# baseline (speedup 1.0000x reference)
"""Trainium2 Bass kernel for the SNN Net (antenna-fuse -> hidden -> LIF scan
-> time-fuse -> output -> softmax), data-parallel over 8 NeuronCores.

Self-contained: hardcodes shapes/sharding; builds the Bass/Tile program and
runs it via run_bass_kernel_spmd.
"""

import os
import sys
from contextlib import ExitStack

import numpy as np

for _p in ("/opt/trn_rl_repo", "/root/.axon_site/_ro/trn_rl_repo"):
    if _p not in sys.path and os.path.isdir(_p):
        sys.path.insert(0, _p)

import concourse.bacc as bacc
import concourse.bass as bass
import concourse.mybir as mybir
import concourse.tile as tile
from concourse.bass_utils import run_bass_kernel_spmd

F32 = mybir.dt.float32
ALU = mybir.AluOpType

B, T, A, D, H, O = 2048, 90, 4, 256, 10, 2
N_CORES = 8
BS = B // N_CORES          # 256 batch rows per core
NB = BS // 128             # 2 row-chunks of 128 partitions
BETA = 0.95
THR = 1.0
TGROUP = 8                 # LIF timesteps per PSUM accumulation bank


def _pick_pairs(w_ant):
    """Order the 4 antennas into two (pivot, other) pairs so that the global
    max-|w| antenna is the pivot of pair A.  Returns indices and the folded
    scalars (rA, rB, cc, base) with |rA|,|rB|,|cc| <= 1 and base = w[pA]."""
    w = np.asarray(w_ant, np.float64)
    order = np.argsort(-np.abs(w))
    pA, aA = int(order[0]), int(order[3])
    pB, aB = int(order[1]), int(order[2])
    # pair B pivot has the 2nd largest |w| -> |w[pB]| >= |w[aB]|
    base = float(w[pA])

    def safe_div(n, d):
        return float(n / d) if abs(d) > 0.0 else 0.0

    rA = safe_div(w[aA], w[pA])
    rB = safe_div(w[aB], w[pB])
    cc = safe_div(w[pB], w[pA])
    return (pA, aA, pB, aB), (rA, rB, cc, base)


def build_program(bs=BS, t_steps=T, gpsimd_stt=True):
    """Build the per-core Bass program.  Returns (nc, scalars_placeholder)
    where scalar immediates are baked in by the caller via the returned
    builder closure -- instead we take the scalars as arguments here."""
    raise NotImplementedError  # replaced below


def _build(nc_scalars, bs, t_steps, gpsimd_stt=False, debug=False):
    """Emit the Bass program.  nc_scalars: dict with rA,rB,cc,b_time floats
    and antenna index order (pA,aA,pB,aB)."""
    rA = nc_scalars["rA"]
    rB = nc_scalars["rB"]
    cc = nc_scalars["cc"]
    b_time = nc_scalars["b_time"]
    pA, aA, pB, aB = nc_scalars["idx"]

    nb = bs // 128
    assert bs % 128 == 0 and nb == 2, "kernel assumes 256 rows/core"
    chw = nb * H                      # 20 free elems per scan step

    # constant blob layout: [128, 2H + 128 + TGROUP*chw]
    #   [:, 0:2H]               wpp  (matmul rhs chunks)
    #   [0, 2H:2H+128]          ones row (bias matmul lhsT)
    #   [0, 2H+128:2H+128+160]  bias_rep (bias matmul rhs)
    blob_w = 2 * H + 128 + TGROUP * chw

    nc = bacc.Bacc()
    x_d = nc.dram_tensor("x", (bs, t_steps, A, D), F32, kind="ExternalInput")
    blob_d = nc.dram_tensor("blob", (128, blob_w), F32, kind="ExternalInput")
    wt_d = nc.dram_tensor("wt_rep", (t_steps * chw,), F32, kind="ExternalInput")
    wout_d = nc.dram_tensor("wout_rep", (O * chw,), F32, kind="ExternalInput")
    out_d = nc.dram_tensor("out", (bs, O), F32, kind="ExternalOutput")
    if debug:
        sn_dbg = nc.dram_tensor("sn_dbg", (t_steps, 128, chw), F32,
                                kind="ExternalOutput")
        spk_dbg = nc.dram_tensor("spk_dbg", (128, t_steps * chw), F32,
                                 kind="ExternalOutput")
        ft_dbg = nc.dram_tensor("ft_dbg", (128, chw), F32,
                                kind="ExternalOutput")

    with ExitStack() as ctx:
        tc = ctx.enter_context(tile.TileContext(nc))
        consts = ctx.enter_context(tc.tile_pool(name="consts", bufs=1))
        xp = ctx.enter_context(tc.tile_pool(name="xp", bufs=6))
        qp = ctx.enter_context(tc.tile_pool(name="qp", bufs=3))
        fp = ctx.enter_context(tc.tile_pool(name="fp", bufs=3))
        ftp = ctx.enter_context(tc.tile_pool(name="ftp", bufs=3))
        state = ctx.enter_context(tc.tile_pool(name="state", bufs=2))
        spkp = ctx.enter_context(tc.tile_pool(name="spk", bufs=1))
        outp = ctx.enter_context(tc.tile_pool(name="outp", bufs=1))
        ps_tr = ctx.enter_context(tc.tile_pool(name="ps_tr", bufs=3, space="PSUM"))
        ps_sn = ctx.enter_context(tc.tile_pool(name="ps_sn", bufs=2, space="PSUM"))

        # ---- constants ----
        ident = consts.tile([128, 128], F32)
        from concourse.masks import make_identity
        make_identity(nc, ident)

        blob = consts.tile([128, blob_w], F32)
        nc.sync.dma_start(out=blob, in_=blob_d[:, :])
        wpp = blob[:, 0:2 * H]
        ones = blob[0:1, 2 * H:2 * H + 128]
        bias_rep = blob[0:1, 2 * H + 128:2 * H + 128 + TGROUP * chw]

        wt_rep = consts.tile([128, t_steps * chw], F32)
        nc.sync.dma_start(
            out=wt_rep,
            in_=bass.AP(tensor=wt_d.tensor if isinstance(wt_d, bass.AP) else wt_d,
                        offset=0, ap=[[0, 128], [1, t_steps * chw]]),
        )
        wout = consts.tile([128, O * chw], F32)
        nc.sync.dma_start(
            out=wout,
            in_=bass.AP(tensor=wout_d.tensor if isinstance(wout_d, bass.AP) else wout_d,
                        offset=0, ap=[[0, 128], [1, O * chw]]),
        )

        spk = spkp.tile([128, t_steps * chw], F32)

        mem = state.tile([128, chw], F32, tag="mem")
        nc.vector.memset(mem, 0.0)

        n_groups = (t_steps + TGROUP - 1) // TGROUP
        for g in range(n_groups):
            ts0 = g * TGROUP
            gsize = min(TGROUP, t_steps - ts0)
            sn = ps_sn.tile([128, TGROUP * chw], F32)
            # bias pre-init of the accumulation bank
            nc.tensor.matmul(
                sn[:, : gsize * chw],
                lhsT=ones,
                rhs=bias_rep[:, : gsize * chw],
                start=True,
                stop=False,
                skip_group_check=True,
            )
            for s in range(gsize):
                t = ts0 + s
                for c in range(nb):
                    x_t = xp.tile([128, A, D], F32, tag="x")
                    nc.sync.dma_start(out=x_t, in_=x_d[c * 128:(c + 1) * 128, t])
                    qA = qp.tile([128, D], F32, tag="qA")
                    nc.vector.scalar_tensor_tensor(
                        out=qA, in0=x_t[:, aA], scalar=rA, in1=x_t[:, pA],
                        op0=ALU.mult, op1=ALU.add)
                    qB = qp.tile([128, D], F32, tag="qB")
                    nc.vector.scalar_tensor_tensor(
                        out=qB, in0=x_t[:, aB], scalar=rB, in1=x_t[:, pB],
                        op0=ALU.mult, op1=ALU.add)
                    fused = fp.tile([128, D], F32, tag="fused")
                    if gpsimd_stt:
                        nc.gpsimd.scalar_tensor_tensor(
                            out=fused, in0=qB, scalar=cc, in1=qA,
                            op0=ALU.mult, op1=ALU.add)
                    else:
                        qBc = fp.tile([128, D], F32, tag="qBc")
                        nc.scalar.mul(qBc, qB, cc)
                        nc.gpsimd.tensor_tensor(
                            out=fused, in0=qBc, in1=qA, op=ALU.add)
                    tr = ps_tr.tile([128, D], F32, tag="tr")
                    nc.tensor.transpose(tr[:, 0:128], fused[:, 0:128], ident)
                    nc.tensor.transpose(tr[:, 128:256], fused[:, 128:256], ident)
                    fT = ftp.tile([128, D], F32, tag="fT")
                    nc.scalar.copy(out=fT, in_=tr)
                    sl = sn[:, s * chw + c * H: s * chw + (c + 1) * H]
                    nc.tensor.matmul(
                        sl, lhsT=fT[:, 0:128], rhs=wpp[:, 0:H],
                        start=False, stop=False, skip_group_check=True)
                    nc.tensor.matmul(
                        sl, lhsT=fT[:, 128:256], rhs=wpp[:, H:2 * H],
                        start=False, stop=True, skip_group_check=True)
            # ---- LIF scan over this group's timesteps ----
            for s in range(gsize):
                t = ts0 + s
                inp = sn[:, s * chw:(s + 1) * chw]
                u = state.tile([128, chw], F32, tag="u")
                nc.vector.scalar_tensor_tensor(
                    out=u, in0=mem, scalar=BETA, in1=inp,
                    op0=ALU.mult, op1=ALU.add)
                mem_new = state.tile([128, chw], F32, tag="mem")
                nc.vector.scalar_tensor_tensor(
                    out=mem_new, in0=mem, scalar=THR, in1=u,
                    op0=ALU.is_le, op1=ALU.mult)
                nc.vector.scalar_tensor_tensor(
                    out=spk[:, t * chw:(t + 1) * chw], in0=mem_new, scalar=THR,
                    in1=wt_rep[:, t * chw:(t + 1) * chw],
                    op0=ALU.is_gt, op1=ALU.mult)
                mem = mem_new
                if debug:
                    dbg = state.tile([128, chw], F32, tag="dbg")
                    nc.vector.tensor_copy(out=dbg, in_=inp)
                    nc.sync.dma_start(out=sn_dbg[t], in_=dbg)

        # ---- time-fuse + output head + softmax ----
        ft = outp.tile([128, chw], F32)
        spk_v = spk[:].rearrange("p (t f) -> p f t", f=chw)
        nc.vector.tensor_reduce(out=ft, in_=spk_v, axis=mybir.AxisListType.X,
                                op=ALU.add)
        nc.vector.tensor_scalar_add(out=ft, in0=ft, scalar1=b_time)
        if debug:
            nc.sync.dma_start(out=spk_dbg[:, :], in_=spk)
            nc.sync.dma_start(out=ft_dbg[:, :], in_=ft)
        lg = outp.tile([128, O * nb], F32)
        for o in range(O):
            mo = outp.tile([128, chw], F32, tag="mo")
            nc.vector.tensor_tensor(out=mo, in0=ft,
                                    in1=wout[:, o * chw:(o + 1) * chw],
                                    op=ALU.mult)
            nc.vector.tensor_reduce(
                out=lg[:, o * nb:(o + 1) * nb],
                in_=mo[:].rearrange("p (c h) -> p c h", h=H),
                axis=mybir.AxisListType.X, op=ALU.add)
            nc.vector.tensor_scalar_add(
                out=lg[:, o * nb:(o + 1) * nb],
                in0=lg[:, o * nb:(o + 1) * nb],
                scalar1=nc_scalars["b_out"][o])
        ex = outp.tile([128, O * nb], F32)
        nc.scalar.activation(out=ex, in_=lg,
                             func=mybir.ActivationFunctionType.Exp)
        ssum = outp.tile([128, nb], F32)
        nc.vector.tensor_tensor(out=ssum, in0=ex[:, 0:nb],
                                in1=ex[:, nb:2 * nb], op=ALU.add)
        rec = outp.tile([128, nb], F32)
        nc.vector.reciprocal(out=rec, in_=ssum)
        res = outp.tile([128, nb * O], F32)
        for c in range(nb):
            for o in range(O):
                nc.vector.tensor_tensor(
                    out=res[:, c * O + o: c * O + o + 1],
                    in0=ex[:, o * nb + c: o * nb + c + 1],
                    in1=rec[:, c: c + 1], op=ALU.mult)
        for c in range(nb):
            nc.sync.dma_start(out=out_d[c * 128:(c + 1) * 128, :],
                              in_=res[:, c * O:(c + 1) * O])
    nc.finalize()
    return nc


def _prep_weights(w_ant, b_ant, w_hid, b_hid, w_time, b_time, w_out, b_out,
                  t_steps=T):
    """Host-side weight folding.  Returns (scalars, const_arrays)."""
    w_ant = np.asarray(w_ant, np.float32)
    w_hid = np.asarray(w_hid, np.float32)
    idx, (rA, rB, cc, base) = _pick_pairs(w_ant)
    chw = NB * H
    # wpp[p, k*H + h] = w_hid[h, k*128+p] * base
    wpp = np.empty((128, 2 * H), np.float32)
    for k in range(2):
        wpp[:, k * H:(k + 1) * H] = (w_hid[:, k * 128:(k + 1) * 128].T
                                     * np.float32(base))
    # bias_rep[s*chw + c*H + h] = b_comb[h]
    b_comb = (np.float32(b_ant) * w_hid.sum(axis=1) + np.asarray(b_hid, np.float32)
              ).astype(np.float32)
    bias_rep = np.tile(b_comb, TGROUP * NB).astype(np.float32)
    blob_w = 2 * H + 128 + TGROUP * chw
    blob = np.zeros((128, blob_w), np.float32)
    blob[:, 0:2 * H] = wpp
    blob[0, 2 * H:2 * H + 128] = 1.0
    blob[0, 2 * H + 128:2 * H + 128 + TGROUP * chw] = bias_rep
    # wt_rep[t*chw + j] = w_time[t]
    wt_rep = np.repeat(np.asarray(w_time, np.float32)[:t_steps], chw
                       ).astype(np.float32)
    # wout_rep[o*chw + c*H + h] = w_out[o, h]
    wout_rep = np.tile(np.asarray(w_out, np.float32), (1, NB)).reshape(-1
                       ).astype(np.float32)
    scalars = {"rA": rA, "rB": rB, "cc": cc, "idx": idx,
               "b_time": float(np.float32(b_time)),
               "b_out": [float(v) for v in np.asarray(b_out, np.float32)]}
    consts = {"blob": blob, "wt_rep": wt_rep, "wout_rep": wout_rep}
    return scalars, consts


_CACHE = {}


def kernel(x, w_ant, b_ant, w_hid, b_hid, w_time, b_time, w_out, b_out):
    x = np.ascontiguousarray(np.asarray(x, np.float32))
    assert x.shape == (B, T, A, D), x.shape
    scalars, consts = _prep_weights(w_ant, b_ant, w_hid, b_hid, w_time,
                                    b_time, w_out, b_out)
    key = (scalars["rA"], scalars["rB"], scalars["cc"], scalars["idx"],
           scalars["b_time"], tuple(scalars["b_out"]))
    nc = _CACHE.get(key)
    if nc is None:
        nc = _build(scalars, BS, T, gpsimd_stt=False)
        _CACHE[key] = nc
    in_maps = []
    for i in range(N_CORES):
        m = {"x": np.ascontiguousarray(x[i * BS:(i + 1) * BS])}
        m.update(consts)
        in_maps.append(m)
    r = run_bass_kernel_spmd(nc, in_maps, core_ids=list(range(N_CORES)))
    out = np.concatenate([r.results[i]["out"] for i in range(N_CORES)], axis=0)
    return out.astype(np.float32)



# revision 4
# speedup vs baseline: 244.0324x; 244.0324x over previous
"""Trainium2 Bass kernel for the SNN Net (antenna-fuse -> hidden -> LIF scan
-> time-fuse -> output -> softmax), data-parallel over 8 NeuronCores.

Self-contained: hardcodes shapes/sharding; builds the Bass/Tile program and
runs it via run_bass_kernel_spmd.

v2 design notes (per core, bs=256 rows = 2 chunks of 128 partitions):
- x is streamed in 10 large DMAs of [128, 18*4096B] (72 KB contiguous per
  partition row) instead of 360 DMAs of 4 KB rows: ~30x fewer descriptors,
  each 18x larger.
- antenna fuse: qA = x[aA]*rA + x[pA], qB = x[aB]*rB + x[pB] on DVE,
  fused = qB*cc + qA on GpSimd; hidden matmul via PE transpose (fp32) of
  fused into PSUM, ACT copy to SBUF, then 2 accumulating matmuls against
  wpp [128d, 10h] plus a K=1 ones-row bias matmul.
- LIF scan on DVE in [128b, 20(c,h)] layout, reading sn straight from PSUM;
  spikes are scaled by w_time[t] via tensor_scalar immediates (no wt_rep
  const tensor needed).
- head: time-reduce + output linear + softmax on DVE/ACT; result is PE-
  transposed to [4,128] so the output DMA is 4 contiguous descriptors.
"""

import os
import sys
from contextlib import ExitStack

import numpy as np

for _p in ("/opt/trn_rl_repo", "/root/.axon_site/_ro/trn_rl_repo"):
    if _p not in sys.path and os.path.isdir(_p):
        sys.path.insert(0, _p)

import concourse.bacc as bacc
import concourse.bass as bass
import concourse.mybir as mybir
import concourse.tile as tile
from concourse.bass_utils import run_bass_kernel_spmd

F32 = mybir.dt.float32
ALU = mybir.AluOpType

B, T, A, D, H, O = 2048, 90, 4, 256, 10, 2
N_CORES = 8
BS = B // N_CORES          # 256 batch rows per core
NB = BS // 128             # 2 row-chunks of 128 partitions
CHW = NB * H               # 20 free elems per scan step
BETA = 0.95
THR = 1.0
TG = 18                    # timesteps per x DMA (72 KB per partition row)
NG = T // TG               # 5 groups
SG = 2                     # timesteps per antenna-fuse DVE op / PSUM pair


def _pick_pairs(w_ant):
    """Order the 4 antennas into two (pivot, other) pairs so the global
    max-|w| antenna is the pivot of pair A. Returns indices and folded
    scalars (rA, rB, cc, base) with |rA|,|rB|,|cc| <= 1 and base = w[pA]."""
    w = np.asarray(w_ant, np.float64)
    order = np.argsort(-np.abs(w))
    pA, aA = int(order[0]), int(order[3])
    pB, aB = int(order[1]), int(order[2])
    base = float(w[pA])

    def safe_div(n, d):
        return float(n / d) if abs(d) > 0.0 else 0.0

    rA = safe_div(w[aA], w[pA])
    rB = safe_div(w[aB], w[pB])
    cc = safe_div(w[pB], w[pA])
    return (pA, aA, pB, aB), (rA, rB, cc, base)


def _build(sc, bs=BS, t_steps=T):
    """Emit the Bass program. sc: dict of host-folded scalars/lists."""
    rA, rB, cc = sc["rA"], sc["rB"], sc["cc"]
    pA, aA, pB, aB = sc["idx"]
    w_time = sc["w_time"]          # list of 90 floats (immediates)
    b_time = sc["b_time"]
    b_out = sc["b_out"]

    nb = bs // 128
    assert bs % 128 == 0 and nb == 2, "kernel assumes 256 rows/core"
    assert t_steps == NG * TG and TG % SG == 0

    nc = bacc.Bacc()
    x_d = nc.dram_tensor("x", (bs, t_steps, A, D), F32, kind="ExternalInput")
    wppT_d = nc.dram_tensor("wppT", (CHW, 128), F32, kind="ExternalInput")
    wb_d = nc.dram_tensor("wb", (1, 64), F32, kind="ExternalInput")
    out_d = nc.dram_tensor("out", (nb * O, 128), F32, kind="ExternalOutput")

    with ExitStack() as ctx:
        tc = ctx.enter_context(tile.TileContext(nc))
        consts = ctx.enter_context(tc.tile_pool(name="consts", bufs=1))
        xp = ctx.enter_context(tc.tile_pool(name="xp", bufs=2))
        qp = ctx.enter_context(tc.tile_pool(name="qp", bufs=2))
        ftp = ctx.enter_context(tc.tile_pool(name="ftp", bufs=3))
        state = ctx.enter_context(tc.tile_pool(name="state", bufs=2))
        spkp = ctx.enter_context(tc.tile_pool(name="spk", bufs=1))
        outp = ctx.enter_context(tc.tile_pool(name="outp", bufs=1))
        ps_ft = ctx.enter_context(tc.tile_pool(name="ps_ft", bufs=3, space="PSUM"))
        ps_sn = ctx.enter_context(tc.tile_pool(name="ps_sn", bufs=2, space="PSUM"))
        ps_ms = ctx.enter_context(tc.tile_pool(name="ps_ms", bufs=1, space="PSUM"))

        # ---- constants ----
        ident = consts.tile([128, 128], F32)
        from concourse.masks import make_identity
        make_identity(nc, ident)

        ones1 = consts.tile([1, 128], F32)
        nc.vector.memset(ones1, 1.0)

        wb = consts.tile([1, 64], F32)
        nc.sync.dma_start(out=wb, in_=wb_d[:, :])
        bcomb = wb[0:1, 0:H]

        wppT = consts.tile([CHW, 128], F32)
        nc.sync.dma_start(out=wppT, in_=wppT_d[:, :])
        # wpp[p, k*H+h] = w_hid[h, k*128+p] * base   (PE transpose of wppT)
        wpp_ps = ps_ms.tile([128, CHW], F32, tag="misc")
        nc.tensor.matmul(wpp_ps, lhsT=wppT, rhs=ident[0:CHW, 0:CHW],
                         is_transpose=True, start=True, stop=True,
                         skip_group_check=True)
        wpp = consts.tile([128, CHW], F32)
        nc.scalar.copy(out=wpp, in_=wpp_ps)

        # broadcast w_out row to all partitions: [128, O*CHW]
        woutb_ps = ps_ms.tile([128, O * CHW], F32, tag="misc")
        nc.tensor.matmul(woutb_ps, lhsT=ones1, rhs=wb[0:1, H:H + O * CHW],
                         start=True, stop=True, skip_group_check=True)
        woutb = consts.tile([128, O * CHW], F32)
        nc.scalar.copy(out=woutb, in_=woutb_ps)

        spk = spkp.tile([128, t_steps * CHW], F32)

        mem = state.tile([128, CHW], F32, tag="mem")
        nc.vector.memset(mem, 0.0)

        sn_tiles = {}
        for g in range(NG):
            for c in range(nb):
                x_t = xp.tile([128, TG, A, D], F32, tag="x")
                src = x_d[c * 128:(c + 1) * 128, g * TG:(g + 1) * TG]
                # alternate the two HWDGE rings (SP / ACT) for the x stream
                dma_eng = nc.sync if (g * nb + c) % 2 == 0 else nc.scalar
                dma_eng.dma_start(out=x_t, in_=src)

                sn = ps_sn.tile([128, TG * H], F32, tag=f"sn{c}")
                sn_tiles[c] = sn
                for sb in range(TG // SG):
                    t0 = sb * SG
                    qA = qp.tile([128, SG, D], F32, tag="qA")
                    nc.vector.scalar_tensor_tensor(
                        out=qA, in0=x_t[:, t0:t0 + SG, aA], scalar=rA,
                        in1=x_t[:, t0:t0 + SG, pA], op0=ALU.mult, op1=ALU.add)
                    qB = qp.tile([128, SG, D], F32, tag="qB")
                    nc.vector.scalar_tensor_tensor(
                        out=qB, in0=x_t[:, t0:t0 + SG, aB], scalar=rB,
                        in1=x_t[:, t0:t0 + SG, pB], op0=ALU.mult, op1=ALU.add)
                    qBc = qp.tile([128, SG, D], F32, tag="qBc")
                    nc.scalar.mul(qBc, qB, cc)
                    fused = qp.tile([128, SG, D], F32, tag="fused")
                    nc.gpsimd.tensor_tensor(out=fused, in0=qBc, in1=qA,
                                            op=ALU.add)
                    # transpose the SG timesteps (2 halves each) into PSUM
                    ftps = ps_ft.tile([128, SG * D], F32, tag="ftps")
                    for tl in range(SG):
                        for h2 in range(2):
                            nc.tensor.matmul(
                                ftps[:, (tl * 2 + h2) * 128:(tl * 2 + h2 + 1) * 128],
                                lhsT=fused[:, tl, h2 * 128:(h2 + 1) * 128],
                                rhs=ident, is_transpose=True,
                                start=True, stop=True, skip_group_check=True)
                    fT = ftp.tile([128, SG * D], F32, tag="fT")
                    nc.scalar.copy(out=fT, in_=ftps)
                    for tl in range(SG):
                        sl = sn[:, (t0 + tl) * H:(t0 + tl + 1) * H]
                        nc.tensor.matmul(sl, lhsT=ones1, rhs=bcomb,
                                         start=True, stop=False,
                                         skip_group_check=True)
                        for h2 in range(2):
                            nc.tensor.matmul(
                                sl,
                                lhsT=fT[:, (tl * 2 + h2) * 128:(tl * 2 + h2 + 1) * 128],
                                rhs=wpp[:, h2 * H:(h2 + 1) * H],
                                start=False, stop=(h2 == 1),
                                skip_group_check=True)
            # ---- LIF scan over this group's timesteps ----
            for tl in range(TG):
                t = g * TG + tl
                u = state.tile([128, CHW], F32, tag="u")
                for c in range(nb):
                    nc.vector.scalar_tensor_tensor(
                        out=u[:, c * H:(c + 1) * H], in0=mem[:, c * H:(c + 1) * H],
                        scalar=BETA, in1=sn_tiles[c][:, tl * H:(tl + 1) * H],
                        op0=ALU.mult, op1=ALU.add)
                mem_new = state.tile([128, CHW], F32, tag="mem")
                nc.vector.scalar_tensor_tensor(
                    out=mem_new, in0=mem, scalar=THR, in1=u,
                    op0=ALU.is_le, op1=ALU.mult)
                nc.vector.tensor_scalar(
                    out=spk[:, t * CHW:(t + 1) * CHW], in0=mem_new,
                    scalar1=THR, scalar2=w_time[t],
                    op0=ALU.is_gt, op1=ALU.mult)
                mem = mem_new

        # ---- time-fuse + output head + softmax ----
        ft = outp.tile([128, CHW], F32)
        spk_v = spk[:].rearrange("p (t f) -> p f t", f=CHW)
        nc.vector.tensor_reduce(out=ft, in_=spk_v, axis=mybir.AxisListType.X,
                                op=ALU.add)
        nc.vector.tensor_scalar_add(out=ft, in0=ft, scalar1=b_time)
        lg = outp.tile([128, O * nb], F32)          # cols o*nb + c
        for o in range(O):
            mo = outp.tile([128, CHW], F32, tag="mo")
            nc.vector.tensor_tensor(out=mo, in0=ft,
                                    in1=woutb[:, o * CHW:(o + 1) * CHW],
                                    op=ALU.mult)
            nc.vector.tensor_reduce(
                out=lg[:, o * nb:(o + 1) * nb],
                in_=mo[:].rearrange("p (c h) -> p c h", h=H),
                axis=mybir.AxisListType.X, op=ALU.add)
            nc.vector.tensor_scalar_add(
                out=lg[:, o * nb:(o + 1) * nb],
                in0=lg[:, o * nb:(o + 1) * nb], scalar1=b_out[o])
        ex = outp.tile([128, O * nb], F32)
        nc.scalar.activation(out=ex, in_=lg,
                             func=mybir.ActivationFunctionType.Exp)
        ssum = outp.tile([128, nb], F32)
        nc.vector.tensor_tensor(out=ssum, in0=ex[:, 0:nb],
                                in1=ex[:, nb:2 * nb], op=ALU.add)
        rec = outp.tile([128, nb], F32)
        nc.vector.reciprocal(out=rec, in_=ssum)
        res = outp.tile([128, nb * O], F32)         # cols c*O + o
        for c in range(nb):
            for o in range(O):
                nc.vector.tensor_tensor(
                    out=res[:, c * O + o: c * O + o + 1],
                    in0=ex[:, o * nb + c: o * nb + c + 1],
                    in1=rec[:, c: c + 1], op=ALU.mult)
        # transpose to [4, 128] so the output DMA is 4 contiguous rows
        resT_ps = ps_ms.tile([nb * O, 128], F32, tag="misc")
        nc.tensor.matmul(resT_ps, lhsT=res, rhs=ident, is_transpose=True,
                         start=True, stop=True, skip_group_check=True)
        resT = outp.tile([nb * O, 128], F32)
        nc.scalar.copy(out=resT, in_=resT_ps)
        nc.sync.dma_start(out=out_d[:, :], in_=resT)
    nc.finalize()
    return nc


def _prep_weights(w_ant, b_ant, w_hid, b_hid, w_time, b_time, w_out, b_out):
    """Host-side weight folding. Returns (scalars, const_arrays)."""
    w_ant = np.asarray(w_ant, np.float32)
    w_hid = np.asarray(w_hid, np.float32)
    w_out = np.asarray(w_out, np.float32)
    idx, (rA, rB, cc, base) = _pick_pairs(w_ant)
    # wppT[k*H+h, p] = w_hid[h, k*128+p] * base
    wppT = np.empty((CHW, 128), np.float32)
    for k in range(NB):
        wppT[k * H:(k + 1) * H, :] = (w_hid[:, k * 128:(k + 1) * 128]
                                      * np.float32(base))
    b_comb = (np.float32(b_ant) * w_hid.sum(axis=1)
              + np.asarray(b_hid, np.float32)).astype(np.float32)
    wb = np.zeros((1, 64), np.float32)
    wb[0, 0:H] = b_comb
    # wout row: [o*CHW + c*H + h] = w_out[o, h]
    wb[0, H:H + O * CHW] = np.concatenate(
        [np.tile(w_out[o], NB) for o in range(O)])
    scalars = {"rA": rA, "rB": rB, "cc": cc, "idx": idx,
               "w_time": [float(v) for v in np.asarray(w_time, np.float32)],
               "b_time": float(np.float32(b_time)),
               "b_out": [float(v) for v in np.asarray(b_out, np.float32)]}
    consts = {"wppT": wppT, "wb": wb}
    return scalars, consts


_CACHE = {}


def kernel(x, w_ant, b_ant, w_hid, b_hid, w_time, b_time, w_out, b_out):
    x = np.ascontiguousarray(np.asarray(x, np.float32))
    assert x.shape == (B, T, A, D), x.shape
    scalars, consts = _prep_weights(w_ant, b_ant, w_hid, b_hid, w_time,
                                    b_time, w_out, b_out)
    key = (scalars["rA"], scalars["rB"], scalars["cc"], scalars["idx"],
           tuple(scalars["w_time"]), scalars["b_time"],
           tuple(scalars["b_out"]))
    nc = _CACHE.get(key)
    if nc is None:
        nc = _build(scalars, BS, T)
        _CACHE[key] = nc
    in_maps = []
    for i in range(N_CORES):
        m = {"x": np.ascontiguousarray(x[i * BS:(i + 1) * BS])}
        m.update(consts)
        in_maps.append(m)
    r = run_bass_kernel_spmd(nc, in_maps, core_ids=list(range(N_CORES)))
    out = np.empty((B, O), np.float32)
    for i in range(N_CORES):
        arr = r.results[i]["out"]          # [nb*O, 128], rows c*O + o
        for c in range(NB):
            blk = arr[c * O:(c + 1) * O, :]            # [O, 128]
            out[i * BS + c * 128:i * BS + (c + 1) * 128, :] = blk.T
    return out
